# revision 14
# baseline (speedup 1.0000x reference)
"""3-layer RGCN (N=20000, E=640000, R=8, H=128, C=16) on 8 Trainium2 cores.

Strategy (dst-sharded message passing):
- Nodes sharded 2560/core (N padded to 20480). Each edge is owned by the core
  that owns its dst node, so segment-sums are core-local (no all-reduce).
- Per layer, per 128-node block nb, per relation r: edges sorted by dst form
  tiles of 128. Each tile: dma_gather of 128 source rows (bf16) feeds a
  scatter matmul  B_r.T += x_g.T @ S  where S[e, n] = norm_e * 1[dst_e == n]
  is a host-precomputed bf16 one-hot-times-norm matrix streamed from DRAM.
- Layer math: agg = sum_r B_r @ W_r + x @ W_root + b  (B_r.T accumulated in
  PSUM, evacuated bf16, then 9 dense matmuls per block accumulate agg.T).
  Layer 0 has no dense stage: gathers read W0_rel[r] rows directly and all
  relations accumulate into one PSUM tile; the root term is a plain add.
- x kept transposed [H, 2560] bf16 in SBUF for the dense stage; row-major
  bf16 copies are produced per block via DMA transpose and AllGather'd so
  every core can gather arbitrary source rows next layer.
- log_softmax over the 16 classes runs on-chip; host concatenates the eight
  [2560, 16] shards and trims to 20000 rows.
"""
import os
import sys
import types
import numpy as np

import concourse.bacc as bacc
import concourse.bass as bass
import concourse.mybir as mybir
import concourse.tile as tile
from concourse.bass_utils import run_bass_kernel_spmd

N = 20000
E = 640000
R = 8
H = 128
C = 16
NCORES = 8
NPAD = 20480
NSH = NPAD // NCORES          # 2560 nodes per core
NBLK = NSH // 128             # 20 node blocks per core
P = 128

F32 = mybir.dt.float32
BF16 = mybir.dt.bfloat16
NP_BF16 = mybir.dt.np(BF16)

LAST_EXEC_TIME_NS = None


def _register_ntff_hook():
    if "antenv.axon_hooks" in sys.modules:
        return
    try:
        from trn_agent_boot.trn_boot import _ntff_profile_via_ctypes
        hook = _ntff_profile_via_ctypes('/opt/axon/libaxon_pjrt.so')
        mod = types.ModuleType("antenv.axon_hooks")
        mod.get_axon_ntff_profile_hook = lambda: hook
        sys.modules["antenv.axon_hooks"] = mod
    except Exception:
        pass


def _roundup(x, m):
    return (x + m - 1) // m * m


def _wrap_idx(idx):
    """[T*128] int array -> [128, T*8] int16 in dma_gather wrapped layout."""
    T = idx.shape[0] // 128
    a = idx.reshape(T, 8, 16).transpose(0, 2, 1)      # [T, 16, 8]
    b = np.tile(a, (1, 8, 1))                          # [T, 128, 8]
    return np.ascontiguousarray(
        b.transpose(1, 0, 2).reshape(128, T * 8)).astype(np.int16)


def _preprocess(edge_index, edge_type):
    """Shared schedule (slot capacities) + per-core packed edge metadata."""
    src = np.asarray(edge_index[0], dtype=np.int64)
    dst = np.asarray(edge_index[1], dtype=np.int64)
    et = np.asarray(edge_type, dtype=np.int64)

    key = dst * R + et
    cnt = np.bincount(key, minlength=N * R)
    norm = (1.0 / np.maximum(cnt[key], 1.0)).astype(np.float64)

    # Degree-balanced node -> (core, block, slot) assignment: deal nodes in
    # descending-degree order round-robin across cores within each block
    # group, so per-(block, relation) edge counts are balanced across cores
    # (capacities are the max over cores, so balance = less padding).
    tot = cnt.reshape(N, R).sum(1)
    rank = np.argsort(-tot, kind="stable")
    allnodes = np.concatenate([rank, np.arange(N, NPAD)])
    j = np.arange(NPAD)
    b_of = j // (NCORES * P)
    c_of = (j % (NCORES * P)) % NCORES
    s_of = (j % (NCORES * P)) // NCORES
    newpos = np.empty(NPAD, np.int64)
    newpos[allnodes] = c_of * NSH + b_of * P + s_of

    dstn = newpos[dst]
    core = dstn // NSH
    nb = (dstn % NSH) // P
    order = np.lexsort((dstn, et, nb, core))
    src_s, dst_s, et_s, norm_s = src[order], dstn[order], et[order], norm[order]
    srcn_s = newpos[src[order]]
    core_s, nb_s = core[order], nb[order]

    counts = np.zeros((NCORES, NBLK, R), np.int64)
    np.add.at(counts, (core_s, nb_s, et_s), 1)
    cap = np.zeros((NBLK, R), np.int64)
    cap16 = np.zeros((NBLK, R), np.int64)
    for b in range(NBLK):
        for r in range(R):
            m = counts[:, b, r].max()
            cap[b, r] = _roundup(m, P) if m > 0 else 0
            cap16[b, r] = _roundup(m, 16) if m > 0 else 0
    tiles_per_slot = (cap // P).astype(np.int64)     # [NBLK, R]
    slot_tile_off = np.zeros((NBLK, R), np.int64)
    t = 0
    for b in range(NBLK):
        for r in range(R):
            slot_tile_off[b, r] = t
            t += tiles_per_slot[b, r]
    TILES = t
    EPAD = TILES * P

    starts = np.zeros(NCORES * NBLK * R, np.int64)
    np.cumsum(counts.ravel()[:-1], out=starts[1:])
    starts = starts.reshape(NCORES, NBLK, R)

    percore = []
    for c in range(NCORES):
        p_src0 = np.zeros(EPAD, np.int64)
        p_src12 = np.zeros(EPAD, np.int64)
        p_dstl = np.zeros(EPAD, np.int64)
        p_norm = np.zeros(EPAD, np.float64)
        for b in range(NBLK):
            for r in range(R):
                if cap[b, r] == 0:
                    continue
                n = counts[c, b, r]
                o = slot_tile_off[b, r] * P
                s0 = starts[c, b, r]
                # sort slot edges by src for HBM read locality (the S
                # matrix encodes dst per edge, so edge order is free)
                o2 = np.argsort(src_s[s0:s0 + n], kind="stable")
                p_src0[o:o + n] = src_s[s0:s0 + n][o2]
                p_src12[o:o + n] = srcn_s[s0:s0 + n][o2]
                p_dstl[o:o + n] = dst_s[s0:s0 + n][o2] % P
                p_norm[o:o + n] = norm_s[s0:s0 + n][o2]
        e = np.arange(EPAD)
        pp = e % P
        tt = e // P
        S = np.zeros((P, TILES * P), np.float32)
        S[pp, tt * P + p_dstl] = p_norm
        percore.append({
            "idx0": _wrap_idx(p_src0),
            "idx12": _wrap_idx(p_src12),
            "S": S.astype(NP_BF16),
        })
    return {
        "cap": cap,
        "tiles_per_slot": tiles_per_slot,
        "slot_tile_off": slot_tile_off,
        "TILES": TILES,
        "percore": percore,
        "newpos": newpos,
        "cap16": cap16,
    }


def _build(sched):
    cap16 = sched["cap16"]
    tiles_per_slot = sched["tiles_per_slot"]
    slot_tile_off = sched["slot_tile_off"]
    TILES = sched["TILES"]
    max_tiles_nb = int(tiles_per_slot.sum(axis=1).max())

    nc = bacc.Bacc("TRN2", target_bir_lowering=False, debug=False,
                   num_devices=NCORES, num_swdge_queues=4,
                   dynamic_dma_scratch_size=49152)

    w0rel = nc.dram_tensor("w0rel", [R, N, H], BF16, kind="ExternalInput")
    w0rootT = nc.dram_tensor("w0rootT", [P, NSH], F32, kind="ExternalInput")
    w1rel = nc.dram_tensor("w1rel", [R, H, H], BF16, kind="ExternalInput")
    w1root = nc.dram_tensor("w1root", [H, H], BF16, kind="ExternalInput")
    b1 = nc.dram_tensor("b1", [H, 1], F32, kind="ExternalInput")
    w2rel = nc.dram_tensor("w2rel", [R, H, C], BF16, kind="ExternalInput")
    w2root = nc.dram_tensor("w2root", [H, C], BF16, kind="ExternalInput")
    b2 = nc.dram_tensor("b2", [C, 1], F32, kind="ExternalInput")
    identC = nc.dram_tensor("identC", [C, C], F32, kind="ExternalInput")
    idx0_d = nc.dram_tensor("idx0", [P, TILES * 8], mybir.dt.int16,
                            kind="ExternalInput")
    idx12_d = nc.dram_tensor("idx12", [P, TILES * 8], mybir.dt.int16,
                             kind="ExternalInput")
    S_d = nc.dram_tensor("S", [P, TILES * P], BF16, kind="ExternalInput")
    out_d = nc.dram_tensor("out", [NSH, C], F32, kind="ExternalOutput")

    qn = [0]
    gcount = [0]

    def next_q():
        q = qn[0]
        qn[0] = (q + 1) % 4
        return q

    with tile.TileContext(nc) as tc:
        with (
            tc.tile_pool(name="meta", bufs=1) as meta,
            tc.tile_pool(name="schunk", bufs=3) as spool,
            tc.tile_pool(name="gbuf", bufs=88) as gpool,
            tc.tile_pool(name="bpool", bufs=4) as bpool,
            tc.tile_pool(name="stage", bufs=4) as stpool,
            tc.tile_pool(name="soft", bufs=2) as softpool,
            tc.tile_pool(name="psA", bufs=2, space="PSUM") as psA,
            tc.tile_pool(name="psB", bufs=4, space="PSUM") as psB,
            tc.tile_pool(name="psC", bufs=2, space="PSUM") as psC,
            tc.tile_pool(name="dram", bufs=1, space="DRAM") as dpool,
        ):
            # ---- resident tiles ----
            idx0_t = meta.tile([P, TILES * 8], mybir.dt.int16, tag="idx0")
            nc.sync.dma_start(out=idx0_t[:], in_=idx0_d[:])
            idx12_t = meta.tile([P, TILES * 8], mybir.dt.int16, tag="idx12")
            nc.sync.dma_start(out=idx12_t[:], in_=idx12_d[:])
            r128 = nc.gpsimd.to_reg(P)
            w0rootT_t = meta.tile([P, NSH], F32, tag="w0rootT")
            nc.sync.dma_start(out=w0rootT_t[:], in_=w0rootT[:])
            w1rel_t = meta.tile([P, R, H], BF16, tag="w1rel")
            nc.sync.dma_start(out=w1rel_t[:],
                              in_=w1rel[:].rearrange("r k m -> k r m"))
            w1root_t = meta.tile([P, H], BF16, tag="w1root")
            nc.sync.dma_start(out=w1root_t[:], in_=w1root[:])
            b1_t = meta.tile([P, 1], F32, tag="b1")
            nc.sync.dma_start(out=b1_t[:], in_=b1[:])
            w2rel_t = meta.tile([P, R, C], BF16, tag="w2rel")
            nc.sync.dma_start(out=w2rel_t[:],
                              in_=w2rel[:].rearrange("r k m -> k r m"))
            w2root_t = meta.tile([P, C], BF16, tag="w2root")
            nc.sync.dma_start(out=w2root_t[:], in_=w2root[:])
            b2_t = meta.tile([C, 1], F32, tag="b2")
            nc.sync.dma_start(out=b2_t[:], in_=b2[:])
            identC_t = meta.tile([C, C], F32, tag="identC")
            nc.sync.dma_start(out=identC_t[:], in_=identC[:])

            xT = [meta.tile([P, NSH], BF16, tag="xT0", name="xT0"),
                  meta.tile([P, NSH], BF16, tag="xT1", name="xT1")]

            x_local = [dpool.tile([NSH, H], BF16, tag="xl0", name="xl0"),
                       dpool.tile([NSH, H], BF16, tag="xl1", name="xl1")]
            x_full = [dpool.tile([NPAD, H], BF16, addr_space="Shared",
                                 tag="xf0", name="xf0"),
                      dpool.tile([NPAD, H], BF16, addr_space="Shared",
                                 tag="xf1", name="xf1")]

            def gather_tile(table_ap, gtile, tglob, idxt, nidx=P):
                # first 2 pool cycles gather full tiles so every slot is
                # fully initialized (trimmed rows read stale SBUF; stale
                # uninitialized bits can be NaN and 0*NaN poisons PSUM)
                if gcount[0] < 96:
                    nidx = P
                gcount[0] += 1
                nc.gpsimd.dma_gather(
                    gtile[:], table_ap,
                    idxt[:, tglob * 8:(tglob + 1) * 8],
                    nidx, r128 if nidx == P else nidx, H,
                    queue_num=next_q(),
                )

            def load_s_chunk(nbk, ntl, t0):
                sch = spool.tile([P, max_tiles_nb * P], BF16, tag="sch")
                nc.sync.dma_start(
                    out=sch[:, :ntl * P], in_=S_d[:, t0 * P:(t0 + ntl) * P])
                return sch

            def stage_x_rows(lyr, nbk):
                stg = stpool.tile([P, H], BF16, tag="stg")
                nc.sync.dma_start(
                    out=stg[:], in_=xT[lyr][:, nbk * P:(nbk + 1) * P],
                    transpose=True)
                nc.sync.dma_start(
                    out=x_local[lyr][:][nbk * P:(nbk + 1) * P, :], in_=stg[:])

            # ================= layer 0 =================
            for nbk in range(NBLK):
                ntl = int(tiles_per_slot[nbk].sum())
                t0 = int(slot_tile_off[nbk, 0])
                if ntl > 0:
                    sch = load_s_chunk(nbk, ntl, t0)
                    acc = psA.tile([P, P], F32, space="PSUM", tag="acc")
                    k = 0
                    for r in range(R):
                        ts0 = int(tiles_per_slot[nbk, r])
                        for jj in range(ts0):
                            nidx = min(P, int(cap16[nbk, r]) - jj * P)
                            g = gpool.tile([P, 1, H], BF16, tag="g")
                            gather_tile(w0rel[r], g, t0 + k, idx0_t, nidx)
                            nc.tensor.matmul(
                                out=acc[:], lhsT=g[:, 0, :],
                                rhs=sch[:, k * P:(k + 1) * P],
                                start=(k == 0), stop=(k == ntl - 1))
                            k += 1
                    tmp = stpool.tile([P, P], F32, tag="tmp0")
                    nc.vector.tensor_tensor(
                        out=tmp[:], in0=acc[:],
                        in1=w0rootT_t[:, nbk * P:(nbk + 1) * P],
                        op=mybir.AluOpType.add)
                else:
                    tmp = stpool.tile([P, P], F32, tag="tmp0")
                    nc.vector.tensor_copy(
                        out=tmp[:], in_=w0rootT_t[:, nbk * P:(nbk + 1) * P])
                nc.scalar.activation(
                    out=xT[0][:, nbk * P:(nbk + 1) * P], in_=tmp[:],
                    func=mybir.ActivationFunctionType.Relu)
                stage_x_rows(0, nbk)

            nc.gpsimd.collective_compute(
                "AllGather", mybir.AluOpType.bypass,
                ins=[x_local[0].opt()], outs=[x_full[0].opt()],
                replica_groups=[list(range(NCORES))])

            # ========== layers 1 and 2 (dense stage) ==========
            def dense_layer(lyr):
                xin = xT[lyr - 1]
                xfull = x_full[lyr - 1]
                wrel_t = w1rel_t if lyr == 1 else w2rel_t
                wroot_t = w1root_t if lyr == 1 else w2root_t
                Mdim = P if lyr == 1 else C
                for nbk in range(NBLK):
                    ntl = int(tiles_per_slot[nbk].sum())
                    t0 = int(slot_tile_off[nbk, 0])
                    if ntl > 0:
                        sch = load_s_chunk(nbk, ntl, t0)
                    rs = [r for r in range(R) if tiles_per_slot[nbk, r] > 0]
                    agg = psA.tile([Mdim, P], F32, space="PSUM", tag="acc")
                    nc.tensor.matmul(
                        out=agg[:], lhsT=wroot_t[:],
                        rhs=xin[:, nbk * P:(nbk + 1) * P],
                        start=True, stop=(len(rs) == 0))
                    k = 0
                    for r in rs:
                        ts = int(tiles_per_slot[nbk, r])
                        bps = psB.tile([P, P], F32, space="PSUM", tag="bps")
                        for j in range(ts):
                            nidx = min(P, int(cap16[nbk, r]) - j * P)
                            g = gpool.tile([P, 1, H], BF16, tag="g")
                            gather_tile(xfull[:], g, t0 + k, idx12_t, nidx)
                            nc.tensor.matmul(
                                out=bps[:], lhsT=g[:, 0, :],
                                rhs=sch[:, k * P:(k + 1) * P],
                                start=(j == 0), stop=(j == ts - 1))
                            k += 1
                        brt = bpool.tile([P, P], BF16, tag="brt")
                        nc.vector.tensor_copy(out=brt[:], in_=bps[:])
                        nc.tensor.matmul(
                            out=agg[:], lhsT=wrel_t[:, r, :], rhs=brt[:],
                            start=False, stop=(r == rs[-1]))
                    yield nbk, agg

            for nbk, agg in dense_layer(1):
                nc.scalar.activation(
                    out=xT[1][:, nbk * P:(nbk + 1) * P], in_=agg[:],
                    func=mybir.ActivationFunctionType.Relu, bias=b1_t[:])
                stage_x_rows(1, nbk)

            nc.gpsimd.collective_compute(
                "AllGather", mybir.AluOpType.bypass,
                ins=[x_local[1].opt()], outs=[x_full[1].opt()],
                replica_groups=[list(range(NCORES))])

            for nbk, agg in dense_layer(2):
                # z = agg + b2 -> [C, P] sbuf f32
                z = softpool.tile([C, P], F32, tag="z")
                nc.scalar.activation(
                    out=z[:], in_=agg[:],
                    func=mybir.ActivationFunctionType.Identity, bias=b2_t[:])
                zt_ps = psC.tile([P, C], F32, space="PSUM", tag="ztps")
                nc.tensor.transpose(out=zt_ps[:], in_=z[:],
                                    identity=identC_t[:])
                zt = softpool.tile([P, C], F32, tag="zt")
                nc.vector.tensor_copy(out=zt[:], in_=zt_ps[:])
                mx = softpool.tile([P, 1], F32, tag="mx")
                nc.vector.reduce_max(out=mx[:], in_=zt[:],
                                     axis=mybir.AxisListType.X)
                nmx = softpool.tile([P, 1], F32, tag="nmx")
                nc.vector.tensor_scalar_mul(nmx[:], mx[:], -1.0)
                ex = softpool.tile([P, C], F32, tag="ex")
                nc.scalar.activation(
                    out=ex[:], in_=zt[:],
                    func=mybir.ActivationFunctionType.Exp, bias=nmx[:])
                sm = softpool.tile([P, 1], F32, tag="sm")
                nc.vector.reduce_sum(out=sm[:], in_=ex[:],
                                     axis=mybir.AxisListType.X)
                lg = softpool.tile([P, 1], F32, tag="lg")
                nc.scalar.activation(
                    out=lg[:], in_=sm[:],
                    func=mybir.ActivationFunctionType.Ln)
                cc = softpool.tile([P, 1], F32, tag="cc")
                nc.vector.tensor_tensor(out=cc[:], in0=lg[:], in1=mx[:],
                                        op=mybir.AluOpType.add)
                res = softpool.tile([P, C], F32, tag="res")
                nc.vector.tensor_scalar(
                    out=res[:], in0=zt[:], scalar1=cc[:], scalar2=None,
                    op0=mybir.AluOpType.subtract)
                nc.sync.dma_start(
                    out=out_d[:][nbk * P:(nbk + 1) * P, :], in_=res[:])

    nc.compile()
    return nc


def kernel(edge_index, edge_type, W0_rel, W0_root, b0, W1_rel, W1_root, b1,
           W2_rel, W2_root, b2):
    global LAST_EXEC_TIME_NS
    trace = os.environ.get("GNN_TRACE", "0") == "1"
    if trace:
        _register_ntff_hook()

    sched = _preprocess(edge_index, edge_type)
    nc = _build(sched)

    W0_rel = np.asarray(W0_rel, np.float32)
    W0_root = np.asarray(W0_root, np.float32)
    b0 = np.asarray(b0, np.float32)
    W1_rel = np.asarray(W1_rel, np.float32)
    W1_root = np.asarray(W1_root, np.float32)
    b1 = np.asarray(b1, np.float32)
    W2_rel = np.asarray(W2_rel, np.float32)
    W2_root = np.asarray(W2_root, np.float32)
    b2 = np.asarray(b2, np.float32)

    newpos = sched["newpos"]
    w0r = W0_root + b0[None, :]
    w0r_pad = np.zeros((NPAD, H), np.float32)
    w0r_pad[newpos[:N]] = w0r

    base = {
        "w0rel": W0_rel.astype(NP_BF16),
        "w1rel": W1_rel.astype(NP_BF16),
        "w1root": W1_root.astype(NP_BF16),
        "b1": b1.reshape(H, 1),
        "w2rel": W2_rel.astype(NP_BF16),
        "w2root": W2_root.astype(NP_BF16),
        "b2": b2.reshape(C, 1),
        "identC": np.eye(C, dtype=np.float32),
    }
    in_maps = []
    for c in range(NCORES):
        pc = sched["percore"][c]
        m = dict(base)
        m["w0rootT"] = np.ascontiguousarray(w0r_pad[c * NSH:(c + 1) * NSH].T)
        m["idx0"] = pc["idx0"]
        m["idx12"] = pc["idx12"]
        m["S"] = pc["S"]
        in_maps.append(m)

    res = run_bass_kernel_spmd(nc, in_maps, list(range(NCORES)), trace=trace)
    LAST_EXEC_TIME_NS = res.exec_time_ns

    out = np.concatenate([res.results[c]["out"] for c in range(NCORES)],
                         axis=0)
    out = out[newpos[:N]]
    return np.ascontiguousarray(out.astype(np.float32))


# revision 16
# speedup vs baseline: 1.0310x; 1.0310x over previous
"""3-layer RGCN (N=20000, E=640000, R=8, H=128, C=16) on 8 Trainium2 cores.

Strategy (dst-sharded message passing):
- Nodes sharded 2560/core (N padded to 20480). Each edge is owned by the core
  that owns its dst node, so segment-sums are core-local (no all-reduce).
- Per layer, per 128-node block nb, per relation r: edges sorted by dst form
  tiles of 128. Each tile: dma_gather of 128 source rows (bf16) feeds a
  scatter matmul  B_r.T += x_g.T @ S  where S[e, n] = norm_e * 1[dst_e == n]
  is a host-precomputed bf16 one-hot-times-norm matrix streamed from DRAM.
- Layer math: agg = sum_r B_r @ W_r + x @ W_root + b  (B_r.T accumulated in
  PSUM, evacuated bf16, then 9 dense matmuls per block accumulate agg.T).
  Layer 0 has no dense stage: gathers read W0_rel[r] rows directly and all
  relations accumulate into one PSUM tile; the root term is a plain add.
- x kept transposed [H, 2560] bf16 in SBUF for the dense stage; row-major
  bf16 copies are produced per block via DMA transpose and AllGather'd so
  every core can gather arbitrary source rows next layer.
- log_softmax over the 16 classes runs on-chip; host concatenates the eight
  [2560, 16] shards and trims to 20000 rows.
"""
import os
import sys
import types
import numpy as np

import concourse.bacc as bacc
import concourse.bass as bass
import concourse.mybir as mybir
import concourse.tile as tile
from concourse.bass_utils import run_bass_kernel_spmd

N = 20000
E = 640000
R = 8
H = 128
C = 16
NCORES = 8
NPAD = 20480
NSH = NPAD // NCORES          # 2560 nodes per core
NBLK = NSH // 128             # 20 node blocks per core
P = 128

F32 = mybir.dt.float32
BF16 = mybir.dt.bfloat16
NP_BF16 = mybir.dt.np(BF16)

LAST_EXEC_TIME_NS = None


def _register_ntff_hook():
    if "antenv.axon_hooks" in sys.modules:
        return
    try:
        from trn_agent_boot.trn_boot import _ntff_profile_via_ctypes
        hook = _ntff_profile_via_ctypes('/opt/axon/libaxon_pjrt.so')
        mod = types.ModuleType("antenv.axon_hooks")
        mod.get_axon_ntff_profile_hook = lambda: hook
        sys.modules["antenv.axon_hooks"] = mod
    except Exception:
        pass


def _roundup(x, m):
    return (x + m - 1) // m * m


def _wrap_idx(idx):
    """[T*128] int array -> [128, T*8] int16 in dma_gather wrapped layout."""
    T = idx.shape[0] // 128
    a = idx.reshape(T, 8, 16).transpose(0, 2, 1)      # [T, 16, 8]
    b = np.tile(a, (1, 8, 1))                          # [T, 128, 8]
    return np.ascontiguousarray(
        b.transpose(1, 0, 2).reshape(128, T * 8)).astype(np.int16)


def _preprocess(edge_index, edge_type):
    """Shared schedule (slot capacities) + per-core packed edge metadata."""
    src = np.asarray(edge_index[0], dtype=np.int64)
    dst = np.asarray(edge_index[1], dtype=np.int64)
    et = np.asarray(edge_type, dtype=np.int64)

    key = dst * R + et
    cnt = np.bincount(key, minlength=N * R)
    norm = (1.0 / np.maximum(cnt[key], 1.0)).astype(np.float64)

    # Degree-balanced node -> (core, block, slot) assignment: deal nodes in
    # descending-degree order round-robin across cores within each block
    # group, so per-(block, relation) edge counts are balanced across cores
    # (capacities are the max over cores, so balance = less padding).
    tot = cnt.reshape(N, R).sum(1)
    rank = np.argsort(-tot, kind="stable")
    allnodes = np.concatenate([rank, np.arange(N, NPAD)])
    deg = np.vstack([cnt.reshape(N, R),
                     np.zeros((NPAD - N, R), np.int64)]).astype(np.float64)
    newpos = np.empty(NPAD, np.int64)
    for g in range(NBLK):
        nodes = allnodes[g * NCORES * P:(g + 1) * NCORES * P]
        percore = np.zeros((NCORES, R), np.float64)
        slots = np.zeros(NCORES, np.int64)
        for n in nodes:
            v = deg[n]
            scores = (percore + v).max(axis=1) + 1e-3 * percore.sum(axis=1)
            scores[slots >= P] = 1e18
            c = int(np.argmin(scores))
            newpos[n] = c * NSH + g * P + slots[c]
            percore[c] += v
            slots[c] += 1

    dstn = newpos[dst]
    core = dstn // NSH
    nb = (dstn % NSH) // P
    order = np.lexsort((dstn, et, nb, core))
    src_s, dst_s, et_s, norm_s = src[order], dstn[order], et[order], norm[order]
    srcn_s = newpos[src[order]]
    core_s, nb_s = core[order], nb[order]

    counts = np.zeros((NCORES, NBLK, R), np.int64)
    np.add.at(counts, (core_s, nb_s, et_s), 1)
    cap = np.zeros((NBLK, R), np.int64)
    cap16 = np.zeros((NBLK, R), np.int64)
    for b in range(NBLK):
        for r in range(R):
            m = counts[:, b, r].max()
            cap[b, r] = _roundup(m, P) if m > 0 else 0
            cap16[b, r] = _roundup(m, 16) if m > 0 else 0
    tiles_per_slot = (cap // P).astype(np.int64)     # [NBLK, R]
    slot_tile_off = np.zeros((NBLK, R), np.int64)
    t = 0
    for b in range(NBLK):
        for r in range(R):
            slot_tile_off[b, r] = t
            t += tiles_per_slot[b, r]
    TILES = t
    EPAD = TILES * P

    starts = np.zeros(NCORES * NBLK * R, np.int64)
    np.cumsum(counts.ravel()[:-1], out=starts[1:])
    starts = starts.reshape(NCORES, NBLK, R)

    percore = []
    for c in range(NCORES):
        p_src0 = np.zeros(EPAD, np.int64)
        p_src12 = np.zeros(EPAD, np.int64)
        p_dstl = np.zeros(EPAD, np.int64)
        p_norm = np.zeros(EPAD, np.float64)
        for b in range(NBLK):
            for r in range(R):
                if cap[b, r] == 0:
                    continue
                n = counts[c, b, r]
                o = slot_tile_off[b, r] * P
                s0 = starts[c, b, r]
                # sort slot edges by src for HBM read locality (the S
                # matrix encodes dst per edge, so edge order is free)
                o2 = np.argsort(src_s[s0:s0 + n], kind="stable")
                p_src0[o:o + n] = src_s[s0:s0 + n][o2]
                p_src12[o:o + n] = srcn_s[s0:s0 + n][o2]
                p_dstl[o:o + n] = dst_s[s0:s0 + n][o2] % P
                p_norm[o:o + n] = norm_s[s0:s0 + n][o2]
        e = np.arange(EPAD)
        pp = e % P
        tt = e // P
        S = np.zeros((P, TILES * P), np.float32)
        S[pp, tt * P + p_dstl] = p_norm
        percore.append({
            "idx0": _wrap_idx(p_src0),
            "idx12": _wrap_idx(p_src12),
            "S": S.astype(NP_BF16),
        })
    return {
        "cap": cap,
        "tiles_per_slot": tiles_per_slot,
        "slot_tile_off": slot_tile_off,
        "TILES": TILES,
        "percore": percore,
        "newpos": newpos,
        "cap16": cap16,
    }


def _build(sched):
    cap16 = sched["cap16"]
    tiles_per_slot = sched["tiles_per_slot"]
    slot_tile_off = sched["slot_tile_off"]
    TILES = sched["TILES"]
    max_tiles_nb = int(tiles_per_slot.sum(axis=1).max())

    nc = bacc.Bacc("TRN2", target_bir_lowering=False, debug=False,
                   num_devices=NCORES, num_swdge_queues=4,
                   dynamic_dma_scratch_size=49152)

    w0rel = nc.dram_tensor("w0rel", [R, N, H], BF16, kind="ExternalInput")
    w0rootT = nc.dram_tensor("w0rootT", [P, NSH], F32, kind="ExternalInput")
    w1rel = nc.dram_tensor("w1rel", [R, H, H], BF16, kind="ExternalInput")
    w1root = nc.dram_tensor("w1root", [H, H], BF16, kind="ExternalInput")
    b1 = nc.dram_tensor("b1", [H, 1], F32, kind="ExternalInput")
    w2rel = nc.dram_tensor("w2rel", [R, H, C], BF16, kind="ExternalInput")
    w2root = nc.dram_tensor("w2root", [H, C], BF16, kind="ExternalInput")
    b2 = nc.dram_tensor("b2", [C, 1], F32, kind="ExternalInput")
    identC = nc.dram_tensor("identC", [C, C], F32, kind="ExternalInput")
    idx0_d = nc.dram_tensor("idx0", [P, TILES * 8], mybir.dt.int16,
                            kind="ExternalInput")
    idx12_d = nc.dram_tensor("idx12", [P, TILES * 8], mybir.dt.int16,
                             kind="ExternalInput")
    S_d = nc.dram_tensor("S", [P, TILES * P], BF16, kind="ExternalInput")
    out_d = nc.dram_tensor("out", [NSH, C], F32, kind="ExternalOutput")

    qn = [0]
    gcount = [0]

    def next_q():
        q = qn[0]
        qn[0] = (q + 1) % 4
        return q

    with tile.TileContext(nc) as tc:
        with (
            tc.tile_pool(name="meta", bufs=1) as meta,
            tc.tile_pool(name="schunk", bufs=3) as spool,
            tc.tile_pool(name="gbuf", bufs=64) as gpool,
            tc.tile_pool(name="bpool", bufs=3) as bpool,
            tc.tile_pool(name="stage", bufs=3) as stpool,
            tc.tile_pool(name="soft", bufs=2) as softpool,
            tc.tile_pool(name="psA", bufs=2, space="PSUM") as psA,
            tc.tile_pool(name="psB", bufs=4, space="PSUM") as psB,
            tc.tile_pool(name="psC", bufs=2, space="PSUM") as psC,
            tc.tile_pool(name="dram", bufs=1, space="DRAM") as dpool,
        ):
            # ---- resident tiles ----
            idx0_t = meta.tile([P, TILES * 8], mybir.dt.int16, tag="idx0")
            nc.sync.dma_start(out=idx0_t[:], in_=idx0_d[:])
            idx12_t = meta.tile([P, TILES * 8], mybir.dt.int16, tag="idx12")
            nc.sync.dma_start(out=idx12_t[:], in_=idx12_d[:])
            r128 = nc.gpsimd.to_reg(P)
            w0rootT_t = meta.tile([P, NSH], F32, tag="w0rootT")
            nc.sync.dma_start(out=w0rootT_t[:], in_=w0rootT[:])
            w1rel_t = meta.tile([P, R, H], BF16, tag="w1rel")
            nc.sync.dma_start(out=w1rel_t[:],
                              in_=w1rel[:].rearrange("r k m -> k r m"))
            w1root_t = meta.tile([P, H], BF16, tag="w1root")
            nc.sync.dma_start(out=w1root_t[:], in_=w1root[:])
            b1_t = meta.tile([P, 1], F32, tag="b1")
            nc.sync.dma_start(out=b1_t[:], in_=b1[:])
            w2rel_t = meta.tile([P, R, C], BF16, tag="w2rel")
            nc.sync.dma_start(out=w2rel_t[:],
                              in_=w2rel[:].rearrange("r k m -> k r m"))
            w2root_t = meta.tile([P, C], BF16, tag="w2root")
            nc.sync.dma_start(out=w2root_t[:], in_=w2root[:])
            b2_t = meta.tile([C, 1], F32, tag="b2")
            nc.sync.dma_start(out=b2_t[:], in_=b2[:])
            identC_t = meta.tile([C, C], F32, tag="identC")
            nc.sync.dma_start(out=identC_t[:], in_=identC[:])

            xT = [meta.tile([P, NSH], BF16, tag="xT0", name="xT0"),
                  meta.tile([P, NSH], BF16, tag="xT1", name="xT1")]

            x_local = [dpool.tile([NSH, H], BF16, tag="xl0", name="xl0"),
                       dpool.tile([NSH, H], BF16, tag="xl1", name="xl1")]
            x_full = [dpool.tile([NPAD, H], BF16, addr_space="Shared",
                                 tag="xf0", name="xf0"),
                      dpool.tile([NPAD, H], BF16, addr_space="Shared",
                                 tag="xf1", name="xf1")]

            def gather_tile(table_ap, gtile, tglob, idxt, nidx=P):
                # first 2 pool cycles gather full tiles so every slot is
                # fully initialized (trimmed rows read stale SBUF; stale
                # uninitialized bits can be NaN and 0*NaN poisons PSUM)
                if gcount[0] < 96:
                    nidx = P
                gcount[0] += 1
                nc.gpsimd.dma_gather(
                    gtile[:], table_ap,
                    idxt[:, tglob * 8:(tglob + 1) * 8],
                    nidx, r128 if nidx == P else nidx, H,
                    queue_num=next_q(),
                )

            def load_s_chunk(nbk, ntl, t0):
                sch = spool.tile([P, max_tiles_nb * P], BF16, tag="sch")
                nc.sync.dma_start(
                    out=sch[:, :ntl * P], in_=S_d[:, t0 * P:(t0 + ntl) * P])
                return sch

            def stage_x_rows(lyr, nbk):
                stg = stpool.tile([P, H], BF16, tag="stg")
                nc.sync.dma_start(
                    out=stg[:], in_=xT[lyr][:, nbk * P:(nbk + 1) * P],
                    transpose=True)
                nc.sync.dma_start(
                    out=x_local[lyr][:][nbk * P:(nbk + 1) * P, :], in_=stg[:])

            # ================= layer 0 =================
            for nbk in range(NBLK):
                ntl = int(tiles_per_slot[nbk].sum())
                t0 = int(slot_tile_off[nbk, 0])
                if ntl > 0:
                    sch = load_s_chunk(nbk, ntl, t0)
                    acc = psA.tile([P, P], F32, space="PSUM", tag="acc")
                    k = 0
                    for r in range(R):
                        ts0 = int(tiles_per_slot[nbk, r])
                        for jj in range(ts0):
                            nidx = min(P, int(cap16[nbk, r]) - jj * P)
                            g = gpool.tile([P, 1, H], BF16, tag="g")
                            gather_tile(w0rel[r], g, t0 + k, idx0_t, nidx)
                            nc.tensor.matmul(
                                out=acc[:], lhsT=g[:, 0, :],
                                rhs=sch[:, k * P:(k + 1) * P],
                                start=(k == 0), stop=(k == ntl - 1))
                            k += 1
                    tmp = stpool.tile([P, P], F32, tag="tmp0")
                    nc.vector.tensor_tensor(
                        out=tmp[:], in0=acc[:],
                        in1=w0rootT_t[:, nbk * P:(nbk + 1) * P],
                        op=mybir.AluOpType.add)
                else:
                    tmp = stpool.tile([P, P], F32, tag="tmp0")
                    nc.vector.tensor_copy(
                        out=tmp[:], in_=w0rootT_t[:, nbk * P:(nbk + 1) * P])
                nc.scalar.activation(
                    out=xT[0][:, nbk * P:(nbk + 1) * P], in_=tmp[:],
                    func=mybir.ActivationFunctionType.Relu)
                stage_x_rows(0, nbk)

            nc.gpsimd.collective_compute(
                "AllGather", mybir.AluOpType.bypass,
                ins=[x_local[0].opt()], outs=[x_full[0].opt()],
                replica_groups=[list(range(NCORES))])

            # ========== layers 1 and 2 (dense stage) ==========
            def dense_layer(lyr):
                xin = xT[lyr - 1]
                xfull = x_full[lyr - 1]
                wrel_t = w1rel_t if lyr == 1 else w2rel_t
                wroot_t = w1root_t if lyr == 1 else w2root_t
                Mdim = P if lyr == 1 else C
                for nbk in range(NBLK):
                    ntl = int(tiles_per_slot[nbk].sum())
                    t0 = int(slot_tile_off[nbk, 0])
                    if ntl > 0:
                        sch = load_s_chunk(nbk, ntl, t0)
                    rs = [r for r in range(R) if tiles_per_slot[nbk, r] > 0]
                    agg = psA.tile([Mdim, P], F32, space="PSUM", tag="acc")
                    nc.tensor.matmul(
                        out=agg[:], lhsT=wroot_t[:],
                        rhs=xin[:, nbk * P:(nbk + 1) * P],
                        start=True, stop=(len(rs) == 0))
                    k = 0
                    for r in rs:
                        ts = int(tiles_per_slot[nbk, r])
                        bps = psB.tile([P, P], F32, space="PSUM", tag="bps")
                        for j in range(ts):
                            nidx = min(P, int(cap16[nbk, r]) - j * P)
                            g = gpool.tile([P, 1, H], BF16, tag="g")
                            gather_tile(xfull[:], g, t0 + k, idx12_t, nidx)
                            nc.tensor.matmul(
                                out=bps[:], lhsT=g[:, 0, :],
                                rhs=sch[:, k * P:(k + 1) * P],
                                start=(j == 0), stop=(j == ts - 1))
                            k += 1
                        brt = bpool.tile([P, P], BF16, tag="brt")
                        nc.vector.tensor_copy(out=brt[:], in_=bps[:])
                        nc.tensor.matmul(
                            out=agg[:], lhsT=wrel_t[:, r, :], rhs=brt[:],
                            start=False, stop=(r == rs[-1]))
                    yield nbk, agg

            for nbk, agg in dense_layer(1):
                nc.scalar.activation(
                    out=xT[1][:, nbk * P:(nbk + 1) * P], in_=agg[:],
                    func=mybir.ActivationFunctionType.Relu, bias=b1_t[:])
                stage_x_rows(1, nbk)

            nc.gpsimd.collective_compute(
                "AllGather", mybir.AluOpType.bypass,
                ins=[x_local[1].opt()], outs=[x_full[1].opt()],
                replica_groups=[list(range(NCORES))])

            for nbk, agg in dense_layer(2):
                # z = agg + b2 -> [C, P] sbuf f32
                z = softpool.tile([C, P], F32, tag="z")
                nc.scalar.activation(
                    out=z[:], in_=agg[:],
                    func=mybir.ActivationFunctionType.Identity, bias=b2_t[:])
                zt_ps = psC.tile([P, C], F32, space="PSUM", tag="ztps")
                nc.tensor.transpose(out=zt_ps[:], in_=z[:],
                                    identity=identC_t[:])
                zt = softpool.tile([P, C], F32, tag="zt")
                nc.vector.tensor_copy(out=zt[:], in_=zt_ps[:])
                mx = softpool.tile([P, 1], F32, tag="mx")
                nc.vector.reduce_max(out=mx[:], in_=zt[:],
                                     axis=mybir.AxisListType.X)
                nmx = softpool.tile([P, 1], F32, tag="nmx")
                nc.vector.tensor_scalar_mul(nmx[:], mx[:], -1.0)
                ex = softpool.tile([P, C], F32, tag="ex")
                nc.scalar.activation(
                    out=ex[:], in_=zt[:],
                    func=mybir.ActivationFunctionType.Exp, bias=nmx[:])
                sm = softpool.tile([P, 1], F32, tag="sm")
                nc.vector.reduce_sum(out=sm[:], in_=ex[:],
                                     axis=mybir.AxisListType.X)
                lg = softpool.tile([P, 1], F32, tag="lg")
                nc.scalar.activation(
                    out=lg[:], in_=sm[:],
                    func=mybir.ActivationFunctionType.Ln)
                cc = softpool.tile([P, 1], F32, tag="cc")
                nc.vector.tensor_tensor(out=cc[:], in0=lg[:], in1=mx[:],
                                        op=mybir.AluOpType.add)
                res = softpool.tile([P, C], F32, tag="res")
                nc.vector.tensor_scalar(
                    out=res[:], in0=zt[:], scalar1=cc[:], scalar2=None,
                    op0=mybir.AluOpType.subtract)
                nc.sync.dma_start(
                    out=out_d[:][nbk * P:(nbk + 1) * P, :], in_=res[:])

    nc.compile()
    return nc


def kernel(edge_index, edge_type, W0_rel, W0_root, b0, W1_rel, W1_root, b1,
           W2_rel, W2_root, b2):
    global LAST_EXEC_TIME_NS
    trace = os.environ.get("GNN_TRACE", "0") == "1"
    if trace:
        _register_ntff_hook()

    sched = _preprocess(edge_index, edge_type)
    nc = _build(sched)

    W0_rel = np.asarray(W0_rel, np.float32)
    W0_root = np.asarray(W0_root, np.float32)
    b0 = np.asarray(b0, np.float32)
    W1_rel = np.asarray(W1_rel, np.float32)
    W1_root = np.asarray(W1_root, np.float32)
    b1 = np.asarray(b1, np.float32)
    W2_rel = np.asarray(W2_rel, np.float32)
    W2_root = np.asarray(W2_root, np.float32)
    b2 = np.asarray(b2, np.float32)

    newpos = sched["newpos"]
    w0r = W0_root + b0[None, :]
    w0r_pad = np.zeros((NPAD, H), np.float32)
    w0r_pad[newpos[:N]] = w0r

    base = {
        "w0rel": W0_rel.astype(NP_BF16),
        "w1rel": W1_rel.astype(NP_BF16),
        "w1root": W1_root.astype(NP_BF16),
        "b1": b1.reshape(H, 1),
        "w2rel": W2_rel.astype(NP_BF16),
        "w2root": W2_root.astype(NP_BF16),
        "b2": b2.reshape(C, 1),
        "identC": np.eye(C, dtype=np.float32),
    }
    in_maps = []
    for c in range(NCORES):
        pc = sched["percore"][c]
        m = dict(base)
        m["w0rootT"] = np.ascontiguousarray(w0r_pad[c * NSH:(c + 1) * NSH].T)
        m["idx0"] = pc["idx0"]
        m["idx12"] = pc["idx12"]
        m["S"] = pc["S"]
        in_maps.append(m)

    res = run_bass_kernel_spmd(nc, in_maps, list(range(NCORES)), trace=trace)
    LAST_EXEC_TIME_NS = res.exec_time_ns

    out = np.concatenate([res.results[c]["out"] for c in range(NCORES)],
                         axis=0)
    out = out[newpos[:N]]
    return np.ascontiguousarray(out.astype(np.float32))
